# revision 1
# baseline (speedup 1.0000x reference)
"""Causal multi-head attention (B=64, T=256, C=384, H=6, D=64) on 8 TRN2 cores.

Strategy: data-parallel over batch (8 batches/core). Per (batch, head) the
attention is computed transposed -- S^T = K Q^T in [s, t] layout. Work is
spread across all four compute engines:

  PE   : QKV matmuls (bf16), scores (bf16, causally-restricted regions),
         AV with interleaved [V|ones] stationary (row sums for free), proj
         (fp32r).
  ACT  : exp, Q-bias evacuations, K1K2/V evacuations.
  DVE  : reciprocal, softmax normalization muls, K0 evacuation, proj
         bias-add evacuations.
  Pool : causal triangle masking (in-place bf16 muls).

Causal structure: for s-chunk0 (s<128) all t>=128 are kept and t<128 is a
triangle; for s-chunk1 only t>=128 exists (triangle). Scores are computed
only on the 384 needed columns per head, and masking touches just two
[128,128] triangles.

Algebraic folds (host-side):
  - K-bias and the q.b_k term cancel in row-softmax -> only Q carries bias,
    and the 1/sqrt(D) scale is folded into W_q and b_q.
  - V-bias passes through attention (softmax rows sum to 1) ->
    b_eff = b_proj + b_v @ W_proj, added during the projection evacuation.
"""
import sys

for _p in ("/opt/trn_rl_repo", "/root/.axon_site/_ro/trn_rl_repo"):
    if _p not in sys.path:
        sys.path.insert(0, _p)

import numpy as np

N_CORES = 8
B, T, C = 64, 256, 384
H, D = 6, 64
BS = B // N_CORES  # batches per core

_compiled = None


def _round_fp32r(x: np.ndarray) -> np.ndarray:
    """Round-to-nearest-even fp32 -> fp32r (11-bit mantissa), matching HW."""
    u = np.ascontiguousarray(x, dtype=np.float32).view(np.uint32).astype(np.uint64)
    lsb = (u >> 12) & 1
    u2 = ((u + 0x7FF + lsb) & 0xFFFFF000).astype(np.uint32)
    return u2.view(np.float32)


def _build():
    import concourse.bass as bass
    import concourse.bacc as bacc
    import concourse.tile as tile
    from concourse import mybir

    F32 = mybir.dt.float32
    F32R = mybir.dt.float32r
    BF16 = mybir.dt.bfloat16
    AF = mybir.ActivationFunctionType

    nc = bacc.Bacc(None)

    xt = nc.dram_tensor("xt", [BS, C, T], BF16, kind="ExternalInput")
    wq = nc.dram_tensor("wq", [C, 3 * C], BF16, kind="ExternalInput")
    wp = nc.dram_tensor("wp", [C, C], F32R, kind="ExternalInput")
    bqs = nc.dram_tensor("bqs", [128, 3], F32, kind="ExternalInput")
    beff = nc.dram_tensor("beff", [128, C], F32, kind="ExternalInput")
    beffr = nc.dram_tensor("beffr", [1, C], F32R, kind="ExternalInput")
    onesr = nc.dram_tensor("onesr", [1, 128], F32R, kind="ExternalInput")
    mk = nc.dram_tensor("mk", [128, 128], BF16, kind="ExternalInput")
    ones_d = nc.dram_tensor("ones_d", [128, C], BF16, kind="ExternalInput")
    y = nc.dram_tensor("y", [BS, T, C], F32, kind="ExternalOutput")

    with tile.TileContext(nc) as tc:
        with (
            tc.tile_pool(name="consts", bufs=1) as consts,
            tc.tile_pool(name="vperm", bufs=1) as vperm,
            tc.tile_pool(name="xts", bufs=4) as p_xts,
            tc.tile_pool(name="qt", bufs=12) as p_qt,
            tc.tile_pool(name="kt", bufs=6) as p_kt,
            tc.tile_pool(name="pr", bufs=8) as p_pr,
            tc.tile_pool(name="rbt", bufs=6) as p_rbt,
            tc.tile_pool(name="yct", bufs=9) as p_yct,
            tc.tile_pool(name="ysb", bufs=4) as p_ysb,
            tc.tile_pool(name="ps_a", bufs=4, space="PSUM") as ps_a,
            tc.tile_pool(name="ps_vy", bufs=2, space="PSUM") as ps_vy,
            tc.tile_pool(name="ps_m", bufs=2, space="PSUM") as ps_m,
        ):
            # ---- constants ----
            # batch-0 x load + QKV weights first: they gate the first matmuls
            xts0 = p_xts.tile([128, 3 * T], BF16, tag="xts", name="xts0")
            nc.gpsimd.dma_start(
                out=xts0, in_=xt[0].rearrange("(j p) t -> p j t", p=128))
            wq_sb = []
            for i in range(3):
                t_ = consts.tile([128, 3 * C], BF16, tag=f"wq{i}")
                wq_sb.append(t_)
            for i in range(3):
                q = nc.sync if i < 2 else nc.gpsimd
                q.dma_start(out=wq_sb[i], in_=wq[i * 128:(i + 1) * 128, :])
            # later-needed consts go on the ACT HWDGE queue (parallel issue)
            bqs_sb = consts.tile([128, 3], F32, tag="bqs")
            nc.sync.dma_start(out=bqs_sb, in_=bqs[:, :])
            # vaug[par][sc]: per 128-col head block: [V_h (64) | ones (64)]
            vaug = [[None, None], [None, None], [None, None]]

            def load_vaug_ones(par, q):
                for sc in range(2):
                    t_ = vperm.tile([128, 6 * 128], BF16, tag=f"vaug{par}{sc}")
                    vaug[par][sc] = t_
                    dst = bass.AP(
                        tensor=t_.tensor,
                        offset=t_[:, :].offset + 64,
                        ap=[t_[:, :].ap[0], [128, 6], [1, 64]],
                    )
                    q.dma_start(out=dst, in_=ones_d[:, 0:C])

            load_vaug_ones(0, nc.sync)
            mk_sb = consts.tile([128, 128], BF16, tag="mk")
            nc.sync.dma_start(out=mk_sb, in_=mk[:, :])
            wp_sb = []
            beff_sb = consts.tile([128, C], F32, tag="beff")
            beffr_sb = consts.tile([1, C], F32R, tag="beffr")
            onesr_sb = consts.tile([1, 128], F32R, tag="onesr")

            def load_late_consts():
                load_vaug_ones(1, nc.sync)
                load_vaug_ones(2, nc.gpsimd)
                for i in range(3):
                    t2 = consts.tile([128, C], F32R, tag=f"wp{i}")
                    nc.sync.dma_start(out=t2, in_=wp[i * 128:(i + 1) * 128, :])
                    wp_sb.append(t2)
                nc.gpsimd.dma_start(out=beff_sb, in_=beff[:, :])
                nc.gpsimd.dma_start(out=beffr_sb, in_=beffr[:, :])
                nc.gpsimd.dma_start(out=onesr_sb, in_=onesr[:, :])

            # ---- per-batch pipeline (piece-interleaved emission) ----
            # Engines execute their streams in emission order, so QKV of
            # batch b+1, heads of batch b and proj of batch b-1 are emitted
            # piece-by-piece per head slot to keep every engine fed.
            state = {}
            proj_state = {}

            def load_xts(b):
                if b >= BS:
                    return
                if b == 0:
                    state[0] = {"xts": xts0}
                    return
                xts = p_xts.tile([128, 3 * T], BF16, tag="xts",
                                 name=f"xts{b}")
                nc.sync.dma_start(
                    out=xts,
                    in_=xt[b].rearrange("(j p) t -> p j t", p=128),
                )
                state[b] = {"xts": xts}

            def qkv_piece(b, piece):
                """piece 0..2: QK pair; piece 3..4: V chunk."""
                if b >= BS or piece >= 5:
                    return
                st = state[b]
                xts = st["xts"]
                par = b % 3
                if piece < 3:
                    # pair p computes (q_p | k_p): unlocks heads 2p, 2p+1
                    p = piece
                    pq = ps_a.tile([128, 2 * T], F32, tag="psa",
                                   name=f"pq{b}_{p}")
                    for half in range(2):
                        for i in range(3):
                            nc.tensor.matmul(
                                pq[:, half * T:(half + 1) * T],
                                wq_sb[i][:, p * 256 + half * 128:
                                          p * 256 + (half + 1) * 128],
                                xts[:, i * T:(i + 1) * T],
                                start=(i == 0),
                                stop=(i == 2),
                            )
                    dst = p_qt.tile([128, T], BF16, tag="qt",
                                    name=f"qt{b}_{p}")
                    nc.scalar.activation(
                        out=dst, in_=pq[:, 0:T], func=AF.Identity,
                        bias=bqs_sb[:, p:p + 1], scale=1.0)
                    st[f"qt{p}"] = dst
                    kdst = p_kt.tile([128, T], BF16, tag="kt",
                                     name=f"kt{b}_{p}")
                    if p == 0:
                        nc.vector.tensor_copy(kdst, pq[:, T:2 * T])
                    else:
                        nc.scalar.activation(out=kdst, in_=pq[:, T:2 * T],
                                             func=AF.Copy)
                    st[f"kt{p}"] = kdst
                else:
                    sc = piece - 3
                    pv = ps_m.tile([128, C], F32, tag="psm",
                                   name=f"pv{b}_{sc}")
                    for i in range(3):
                        nc.tensor.matmul(
                            pv,
                            xts[:, i * T + sc * 128:i * T + (sc + 1) * 128],
                            wq_sb[i][:, 2 * C:3 * C],
                            start=(i == 0),
                            stop=(i == 2),
                        )
                    # scatter V head-dims into vaug blocks (cols 128h..+64)
                    vt = vaug[par][sc]
                    dst = bass.AP(
                        tensor=vt.tensor, offset=vt[:, :].offset,
                        ap=[vt[:, :].ap[0], [128, 6], [1, 64]],
                    )
                    srcap = bass.AP(
                        tensor=pv.tensor, offset=pv[:, :].offset,
                        ap=[pv[:, :].ap[0], [64, 6], [1, 64]],
                    )
                    nc.scalar.activation(out=dst, in_=srcap, func=AF.Copy)

            def heads_piece(b, h):
                par = b % 3
                st = state[b]
                if h == 0:
                    st["yct"] = [
                        p_yct.tile([128, T], F32R, tag="yct",
                                   name=f"yct{b}_{j}")
                        for j in range(3)
                    ]
                rb = 64 * (h % 2)
                kh = st[f"kt{h // 2}"][rb:rb + 64, :]
                qh = st[f"qt{h // 2}"][rb:rb + 64, :]

                pst = ps_a.tile([128, 2 * T], F32, tag="psa",
                                name=f"pst{b}_{h}")
                # s-chunk0 x all t (256 wide), s-chunk1 x t>=128 (128)
                nc.tensor.matmul(
                    pst[:, 0:T], kh[:, 0:128], qh, start=True, stop=True)
                nc.tensor.matmul(
                    pst[:, T:T + 128], kh[:, 128:256], qh[:, 128:T],
                    start=True, stop=True)
                pr = p_pr.tile([128, T + 128], BF16, tag="pr",
                               name=f"pr{b}_{h}")
                nc.scalar.activation(out=pr, in_=pst[:, 0:T + 128],
                                     func=AF.Exp)
                # causal triangles (in-place, Pool engine)
                nc.gpsimd.tensor_mul(pr[:, 0:128], pr[:, 0:128], mk_sb)
                nc.gpsimd.tensor_mul(pr[:, T:T + 128], pr[:, T:T + 128],
                                     mk_sb)

                col = 256 * (h % 2)
                if h % 2 == 0:
                    st["pvy"] = ps_vy.tile([128, 2 * T], F32, tag="vy",
                                           name=f"pvy{b}_{h // 2}")
                pvy = st["pvy"]
                nc.tensor.matmul(
                    pvy[:, col:col + T],
                    vaug[par][0][:, h * 128:(h + 1) * 128],
                    pr[:, 0:T],
                    start=True,
                    stop=True,
                )
                nc.tensor.matmul(
                    pvy[:, col + 128:col + T],
                    vaug[par][1][:, h * 128:(h + 1) * 128],
                    pr[:, T:T + 128],
                    start=False,
                    stop=True,
                )
                if h % 2 == 1:
                    i = h // 2
                    yct = st["yct"]
                    rbt = p_rbt.tile([64, 2 * T], F32, tag="rbt",
                                     name=f"rbt{b}_{i}")
                    with nc.allow_low_precision(reason="softmax recip"):
                        nc.vector.reciprocal(
                            out=rbt, in_=pvy[64:128, :])
                        nc.vector.tensor_mul(
                            yct[i][0:64, :], pvy[0:64, 0:T],
                            rbt[:, 0:T])
                        nc.vector.tensor_mul(
                            yct[i][64:128, :], pvy[0:64, T:2 * T],
                            rbt[:, T:2 * T])

            def proj_piece(b, piece):
                """piece 0/2: py matmuls; 1/3: bias-add evac; 4: y DMA."""
                if b < 0 or b >= BS or (piece >= 4 and piece < 10):
                    return
                if piece == 0:
                    proj_state[b] = {
                        "ysb": p_ysb.tile([128, 2 * C], F32,
                                          tag="ysb", name=f"ysb{b}"),
                    }
                ps = proj_state.get(b)
                yct = state[b]["yct"]
                if piece in (0, 2):
                    tck = piece // 2
                    last = b == BS - 1
                    py = ps_m.tile([128, C], F32, tag="psm",
                                   name=f"py{b}_{tck}")
                    if last:  # bias folded in as a 1-partition matmul
                        nc.tensor.matmul(py, onesr_sb, beffr_sb,
                                         start=True, stop=False)
                    for j in range(3):
                        nc.tensor.matmul(
                            py,
                            yct[j][:, tck * 128:(tck + 1) * 128],
                            wp_sb[j][:, :],
                            start=False if last else (j == 0),
                            stop=(j == 2),
                        )
                    ps[f"py{tck}"] = py
                elif piece in (1, 3):
                    tck = piece // 2
                    if b == BS - 1:  # bias already in PSUM; parallel evacs
                        ev = nc.scalar.activation if tck == 0 else None
                        if tck == 0:
                            nc.scalar.activation(
                                out=ps["ysb"][:, 0:C],
                                in_=ps["py0"], func=AF.Copy)
                        else:
                            nc.vector.tensor_copy(
                                ps["ysb"][:, C:2 * C], ps["py1"])
                    else:
                        nc.vector.tensor_add(
                            ps["ysb"][:, tck * C:(tck + 1) * C],
                            ps[f"py{tck}"], beff_sb)
                    nc.sync.dma_start(
                        out=y[b, tck * 128:(tck + 1) * 128, :],
                        in_=ps["ysb"][:, tck * C:(tck + 1) * C],
                    )
                    if piece == 3:
                        del proj_state[b]
                        del state[b]
            # prologue: batch 0 QKV fully, then interleaved main loop
            load_xts(0)
            load_xts(1)
            for piece in range(5):
                qkv_piece(0, piece)
            load_late_consts()
            for b in range(BS):
                last = b == BS - 1
                for h in range(6):
                    qkv_piece(b + 1, h)       # pieces 0-4; h==5 no-op
                    heads_piece(b, h)
                    if h == 5:
                        load_xts(b + 2)
                    proj_piece(b - 1, h)      # pieces 0-4; h==5 no-op
            for piece in range(4):
                proj_piece(BS - 1, piece)

    nc.compile()
    return nc


def _get_compiled():
    global _compiled
    if _compiled is None:
        _compiled = _build()
    return _compiled


def _make_in_maps(x, W_qkv, b_qkv, W_proj, b_proj):
    import ml_dtypes

    x = np.asarray(x, dtype=np.float32)
    W_qkv = np.asarray(W_qkv, dtype=np.float32)
    b_qkv = np.asarray(b_qkv, dtype=np.float32)
    W_proj = np.asarray(W_proj, dtype=np.float32)
    b_proj = np.asarray(b_proj, dtype=np.float32)

    wq_mod = W_qkv.copy()
    wq_mod[:, :C] *= 0.125                      # fold attn scale into W_q
    # column reorder: [q0|k0|q1|k1|q2|k2|V] so each pair loads contiguously
    cols = []
    for p in range(3):
        cols.extend(range(p * 128, (p + 1) * 128))          # q_p
        cols.extend(range(C + p * 128, C + (p + 1) * 128))  # k_p
    cols.extend(range(2 * C, 3 * C))                        # V
    wq_bf = wq_mod[:, cols].astype(ml_dtypes.bfloat16)
    wp_r = _round_fp32r(W_proj)
    bqs = np.ascontiguousarray(
        (0.125 * b_qkv[:C]).reshape(3, 128).T, dtype=np.float32
    )
    beff_row = (b_proj + b_qkv[2 * C:] @ W_proj).astype(np.float32)
    beff = np.ascontiguousarray(np.broadcast_to(beff_row, (128, C)))
    beffr = _round_fp32r(beff_row.reshape(1, C))
    onesr = np.ones((1, 128), dtype=np.float32)
    idx = np.arange(128)
    mk = (idx[None, :] >= idx[:, None]).astype(ml_dtypes.bfloat16)
    mk = np.ascontiguousarray(mk)
    ones_d = np.ones((128, C), dtype=ml_dtypes.bfloat16)

    in_maps = []
    for c in range(N_CORES):
        xs = x[c * BS:(c + 1) * BS]                      # [BS, T, C]
        xtr = np.ascontiguousarray(
            xs.transpose(0, 2, 1)).astype(ml_dtypes.bfloat16)
        in_maps.append({
            "xt": xtr, "wq": wq_bf, "wp": wp_r, "bqs": bqs,
            "beff": beff, "mk": mk, "ones_d": ones_d,
            "beffr": beffr, "onesr": onesr,
        })
    return in_maps


def kernel(x, W_qkv, b_qkv, W_proj, b_proj):
    nc = _get_compiled()
    from concourse.bass_utils import run_bass_kernel_spmd

    in_maps = _make_in_maps(x, W_qkv, b_qkv, W_proj, b_proj)
    res = run_bass_kernel_spmd(nc, in_maps, core_ids=list(range(N_CORES)))
    out = np.concatenate([res.results[c]["y"] for c in range(N_CORES)], axis=0)
    return out.astype(np.float32)



# revision 35
# speedup vs baseline: 1.1344x; 1.1344x over previous
"""Causal multi-head attention (B=64, T=256, C=384, H=6, D=64) on 8 TRN2 cores.

Data-parallel over batch (8 batches/core). Attention computed transposed
per (batch, head): S^T = K Q^T in [t, s] layout.

v2 design (fp8 DoubleRow):
  PE   : all matmuls in fp8e4m3 with DoubleRow perf mode (2 K-subtiles per
         pass, 0.5 cyc/row) except the projection (bf16 for accuracy).
         Causal masking is done ON the PE: a tiny constant matmul
         (I^T @ (-240*TRI)) accumulates -240 into the masked triangle of
         the score PSUM before exp. QKV biases ride the contraction as a
         4th "ones-row" chunk; the projection bias rides as a K=1 matmul.
  ACT  : exp (scale=0.125 folds the attention scale), v-scatter into the
         AV stationary layout, pair-2 QK evac, proj evacuation (bf16 y).
  DVE  : pair-0/1 QK evac, pvy evacuation, reciprocal of softmax sums.
  Pool : softmax normalization multiplies (SBUF-only engine).

Softmax denominators ride the AV matmul via ones-columns interleaved with
V in the stationary ([V_h | ones] per head), yielding row sums on
partitions 64:128 of the AV output.

PSUM (8 banks): ring A = 3 x [128,1024] shared by {pq01, pq2, pst x3,
pvy x3}; ring B = 1 x [128,1024] shared by {pv, py} (proj evac emitted
before the V matmuls so the buffer frees in time).
"""
import sys

for _p in ("/opt/trn_rl_repo", "/root/.axon_site/_ro/trn_rl_repo"):
    if _p not in sys.path:
        sys.path.insert(0, _p)

import numpy as np

N_CORES = 8
B, T, C = 64, 256, 384
H, D = 6, 64
BS = B // N_CORES  # batches per core

_compiled = None


def _build():
    import concourse.bass as bass
    import concourse.bacc as bacc
    import concourse.tile as tile
    from concourse import mybir

    F32 = mybir.dt.float32
    F8 = mybir.dt.float8e4
    BF16 = mybir.dt.bfloat16
    AF = mybir.ActivationFunctionType
    DR = mybir.MatmulPerfMode.DoubleRow

    nc = bacc.Bacc(None)

    # DRAM tensors
    xt = nc.dram_tensor("xt", [BS, 512, T], F8, kind="ExternalInput")
    xtb = nc.dram_tensor("xtb", [BS, 384, T], BF16, kind="ExternalInput")
    wq = nc.dram_tensor("wq", [512, 2 * C], F8, kind="ExternalInput")
    wv = nc.dram_tensor("wv", [3 * 128, C], BF16, kind="ExternalInput")
    wp = nc.dram_tensor("wp", [3 * 128, C], BF16, kind="ExternalInput")
    cpack = nc.dram_tensor("cpack", [128, 512], F8, kind="ExternalInput")
    onesb = nc.dram_tensor("onesb", [128, 768], BF16, kind="ExternalInput")
    bpack = nc.dram_tensor("bpack", [1, 512], BF16, kind="ExternalInput")
    y = nc.dram_tensor("y", [BS, T, C], BF16, kind="ExternalOutput")

    QKW = 640   # per-pair block in qk_sb: q(256) | k(256) | zeros(128)
    VAW = 1664  # vaug: 2 * 768 (sc-major, 6 x [V|ones]) + zeros(128)

    def mkap(t_, rb, np_, col, dims):
        """AP over tile t_ at partition base rb (np_ partitions), free
        offset col, extra free dims `dims` ([stride, count] pairs)."""
        full = t_[:, :]
        pstr = full.ap[0][0]
        return bass.AP(
            tensor=t_.tensor,
            offset=full.offset + rb * pstr + col,
            ap=[[pstr, np_]] + dims,
        )

    with tile.TileContext(nc) as tc:
        with (
            tc.tile_pool(name="consts", bufs=1) as consts,
            tc.tile_pool(name="xts", bufs=4) as p_xts,
            tc.tile_pool(name="qk", bufs=2) as p_qk,
            tc.tile_pool(name="pr", bufs=6) as p_pr,
            tc.tile_pool(name="vaug", bufs=2) as p_vaug,
            tc.tile_pool(name="pvysb", bufs=6) as p_pvysb,
            tc.tile_pool(name="rbt", bufs=6) as p_rbt,
            tc.tile_pool(name="yct", bufs=3) as p_yct,
            tc.tile_pool(name="ysb", bufs=3) as p_ysb,
            tc.tile_pool(name="psP", bufs=2, space="PSUM") as ps_p,
            tc.tile_pool(name="psS", bufs=2, space="PSUM") as ps_s,
            tc.tile_pool(name="psB", bufs=1, space="PSUM") as ps_b,
        ):
            # ---- constants ----
            xts0 = p_xts.tile([128, 4 * T], F8, tag="xts", name="xts0")
            nc.sync.dma_start(
                out=xts0, in_=xt[0].rearrange("(j p) t -> p j t", p=128))
            wq_sb = consts.tile([128, 4 * 768], F8, tag="wq")
            for i in range(2):
                nc.scalar.dma_start(
                    out=wq_sb[:, i * 1536:(i + 1) * 1536],
                    in_=wq[i * 256:(i + 1) * 256].rearrange(
                        "(j p) c -> p j c", p=128))
            xtb0 = p_xts.tile([128, 3 * T], BF16, tag="xtb", name="xtb0")
            nc.sync.dma_start(
                out=xtb0, in_=xtb[0].rearrange("(j p) t -> p j t", p=128))
            cpack_sb = consts.tile([128, 512], F8, tag="cpack")
            nc.scalar.dma_start(out=cpack_sb, in_=cpack[:, :])
            maskc_sb = cpack_sb[:, 0:256]
            trim_sb = cpack_sb[:, 256:512]
            wv_sb = consts.tile([128, 3 * C], BF16, tag="wv")
            wp_sb = consts.tile([128, 3 * C], BF16, tag="wp")
            bpack_sb = consts.tile([1, 512], BF16, tag="bpack")
            beffr_sb = bpack_sb[:, 0:384]
            onesr_sb = bpack_sb[:, 384:512]

            def load_late_consts():
                nc.scalar.dma_start(out=wv_sb, in_=wv.rearrange(
                    "(j p) c -> p j c", p=128))
                nc.scalar.dma_start(out=wp_sb, in_=wp.rearrange(
                    "(j p) c -> p j c", p=128))
                nc.scalar.dma_start(out=bpack_sb, in_=bpack[:, :])

            # fixed double-buffers with constant regions initialized once
            qk_bufs = [[p_qk.tile([128, QKW], F8, tag=f"qk{p}",
                                  name=f"qkbuf{p}_{i}") for p in range(3)]
                       for i in range(2)]
            for bufs in qk_bufs:
                for t_ in bufs:
                    nc.gpsimd.memzero(t_[:, 512:QKW])
            vaug_bufs = [p_vaug.tile([128, VAW], BF16, tag="vaug",
                                     name=f"vabuf{i}") for i in range(2)]
            for i, t_ in enumerate(vaug_bufs):
                nc.gpsimd.memzero(t_[:, 1536:1664])
                dst = mkap(t_, 0, 128, 64, [[768, 2], [128, 6], [1, 64]])
                q = nc.sync if i == 0 else nc.scalar
                q.dma_start(out=dst, in_=onesb[:, :])

            # PE p-state warmup: dummy matmuls on a zeroed scratch so the
            # tensor engine reaches full clock before real work arrives.
            warm_sb = consts.tile([128, 512], F8, tag="warm")
            nc.gpsimd.memzero(warm_sb[:, :])
            pwarm = ps_s.tile([128, 512], F32, tag="S", name="pwarm")
            for _w in range(10):
                nc.tensor.matmul(pwarm[:, 0:512], warm_sb[:, 0:128],
                                 warm_sb[:, 0:512],
                                 start=True, stop=True,
                                 skip_group_check=True)

            state = {}

            def qk_tile(b):
                return qk_bufs[b % 2]

            def vaug_tile(b):
                return vaug_bufs[b % 2]

            def load_xts(b):
                if b >= BS:
                    return
                if b == 0:
                    state[0] = {"xts": xts0, "xtb": xtb0}
                    return
                xts = p_xts.tile([128, 4 * T], F8, tag="xts", name=f"xts{b}")
                nc.sync.dma_start(
                    out=xts, in_=xt[b].rearrange("(j p) t -> p j t", p=128))
                xtbt = p_xts.tile([128, 3 * T], BF16, tag="xtb",
                                  name=f"xtb{b}")
                nc.sync.dma_start(
                    out=xtbt, in_=xtb[b].rearrange("(j p) t -> p j t", p=128))
                state[b] = {"xts": xts, "xtb": xtbt}

            def qkv_piece(b, piece):
                if b >= BS:
                    return
                st = state[b]
                xts = st["xts"]
                qk = qk_tile(b)
                if piece == 0:
                    # pairs 0,1 -> two 1-bank tiles; DVE evacs
                    for p in range(2):
                        pq = ps_s.tile([128, 512], F32, tag="S",
                                       name=f"pq{p}_{b}")
                        for half in range(2):
                            col = half * 256
                            wcol = p * 256 + half * 128
                            for g in range(2):
                                nc.tensor.matmul(
                                    pq[:, col:col + 256],
                                    mkap(wq_sb, 0, 128, g * 2 * 768 + wcol,
                                         [[768, 2], [1, 128]]),
                                    mkap(xts, 0, 128, g * 2 * T,
                                         [[T, 2], [1, T]]),
                                    start=(g == 0), stop=(g == 1),
                                    perf_mode=DR, skip_group_check=True,
                                )
                        nc.vector.tensor_copy(qk[p][:, 0:512], pq)
                elif piece == 1:
                    # pair 2 -> 1-bank tile; ACT evac
                    pq = ps_s.tile([128, 512], F32, tag="S", name=f"pq2_{b}")
                    for half in range(2):
                        col = half * 256
                        wcol = 512 + half * 128
                        for g in range(2):
                            nc.tensor.matmul(
                                pq[:, col:col + 256],
                                mkap(wq_sb, 0, 128, g * 2 * 768 + wcol,
                                     [[768, 2], [1, 128]]),
                                mkap(xts, 0, 128, g * 2 * T,
                                     [[T, 2], [1, T]]),
                                start=(g == 0), stop=(g == 1),
                                perf_mode=DR, skip_group_check=True,
                            )
                    nc.scalar.activation(
                        out=qk[2][:, 0:512], in_=pq[:, 0:512], func=AF.Copy)
                elif piece == 2:
                    # V [t-chunk, 384] bf16 for sc=0,1 into B tile
                    xtbt = st["xtb"]
                    pv = ps_b.tile([128, 1024], F32, tag="B", name=f"pv{b}")
                    st["pv"] = pv
                    for sc in range(2):
                        for j in range(3):
                            nc.tensor.matmul(
                                pv[:, sc * 512:sc * 512 + 384],
                                xtbt[:, j * T + sc * 128:j * T + sc * 128
                                     + 128],
                                wv_sb[:, j * C:(j + 1) * C],
                                start=(j == 0), stop=(j == 2),
                                skip_group_check=True,
                            )
                elif piece == 3:
                    # v scatter (ACT): pv [128, 2, 6, 64] -> vaug [V|ones]
                    pv = st["pv"]
                    va = vaug_tile(b)
                    src = mkap(pv, 0, 128, 0, [[512, 2], [64, 6], [1, 64]])
                    dst = mkap(va, 0, 128, 0, [[768, 2], [128, 6], [1, 64]])
                    nc.scalar.activation(out=dst, in_=src, func=AF.Copy)
                    del st["pv"]

            def heads_piece(b, g, sub):
                """Head group g (heads 2g, 2g+1); sub 0: scores+mask+exp,
                sub 1: AV + evac + recip + muls."""
                if b < 0 or b >= BS:
                    return
                st = state[b]
                qk = qk_tile(b)[g]
                va = vaug_tile(b)
                if sub == 0:
                    pst = ps_p.tile([128, 1024], F32, tag="P",
                                    name=f"pst{b}_{g}")
                    st[f"pst{g}"] = pst
                    msta = bass.AP(
                        tensor=cpack_sb.tensor,
                        offset=maskc_sb.offset,
                        ap=[[maskc_sb.ap[0][0], 128], [128, 2], [1, 128]],
                    )
                    mmov = bass.AP(
                        tensor=cpack_sb.tensor,
                        offset=trim_sb.offset,
                        ap=[[trim_sb.ap[0][0], 128], [128, 2], [1, 128]],
                    )
                    for hh in range(2):
                        rb = 64 * hh
                        col = hh * 512
                        for tck in range(2):
                            n = 256 if tck == 0 else 128
                            ksta = mkap(qk, rb, 64,
                                        256 + tck * 128,
                                        [[256 - tck * 128, 2], [1, 128]])
                            qmov = mkap(qk, rb, 64,
                                        (0 if tck == 0 else 128),
                                        [[0, 2], [1, n]])
                            nc.tensor.matmul(
                                pst[:, col + tck * 256:col + tck * 256 + n],
                                ksta, qmov,
                                start=True, stop=False,
                                perf_mode=DR, skip_group_check=True,
                            )
                            nc.tensor.matmul(
                                pst[:, col + tck * 256:col + tck * 256 + 128],
                                msta, mmov,
                                start=False, stop=True,
                                perf_mode=DR, skip_group_check=True,
                            )
                    pr = p_pr.tile([128, 768], BF16, tag="pr",
                                   name=f"pr{b}_{g}")
                    st[f"pr{g}"] = pr
                    nc.scalar.activation(
                        out=mkap(pr, 0, 128, 0, [[384, 2], [1, 384]]),
                        in_=mkap(pst, 0, 128, 0, [[512, 2], [1, 384]]),
                        func=AF.Exp, scale=0.125)
                    del st[f"pst{g}"]
                else:
                    pr = st[f"pr{g}"]
                    pvy = ps_s.tile([128, 512], F32, tag="S",
                                    name=f"pvy{b}_{g}")
                    for hh in range(2):
                        h = 2 * g + hh
                        col = hh * 256
                        # s-chunk 0: contract t0 only (bf16)
                        nc.tensor.matmul(
                            pvy[:, col:col + 128],
                            va[:, h * 128:h * 128 + 128],
                            pr[:, hh * 384:hh * 384 + 128],
                            start=True, stop=True,
                            skip_group_check=True,
                        )
                        # s-chunk 1: accumulate t0 then t1 (bf16)
                        nc.tensor.matmul(
                            pvy[:, col + 128:col + 256],
                            va[:, h * 128:h * 128 + 128],
                            pr[:, hh * 384 + 128:hh * 384 + 256],
                            start=True, stop=False,
                            skip_group_check=True,
                        )
                        nc.tensor.matmul(
                            pvy[:, col + 128:col + 256],
                            va[:, 768 + h * 128:768 + h * 128 + 128],
                            pr[:, hh * 384 + 256:hh * 384 + 384],
                            start=False, stop=True,
                            skip_group_check=True,
                        )
                    pvysb = p_pvysb.tile([128, 512], F32, tag="pvysb",
                                         name=f"pvysb{b}_{g}")
                    st[f"pvysb{g}"] = pvysb
                    nc.vector.tensor_copy(pvysb, pvy)
                    del st[f"pr{g}"]

            def heads_norm(b, g):
                """Reciprocal + normalization muls for head group g."""
                if b < 0 or b >= BS:
                    return
                st = state[b]
                pvysb = st[f"pvysb{g}"]
                rbt = p_rbt.tile([64, 512], F32, tag="rbt",
                                 name=f"rbt{b}_{g}")
                with nc.allow_low_precision(reason="softmax recip"):
                    nc.vector.reciprocal(out=rbt, in_=pvysb[64:128, :])
                if g == 0:
                    st["yct"] = [
                        p_yct.tile([128, T], BF16, tag=f"yct{j}",
                                   name=f"yct{b}_{j}")
                        for j in range(3)
                    ]
                yct = st["yct"][g]
                nc.gpsimd.tensor_mul(
                    yct[0:64, :], pvysb[0:64, 0:256], rbt[:, 0:256])
                nc.gpsimd.tensor_mul(
                    yct[64:128, :], pvysb[0:64, 256:512], rbt[:, 256:512])
                del st[f"pvysb{g}"]

            def proj_open(b):
                """Allocate py and ride the bias in (both tck halves)."""
                if b < 0 or b >= BS:
                    return
                st = state[b]
                py = ps_b.tile([128, 1024], F32, tag="B", name=f"py{b}")
                st["py"] = py
                for tck in range(2):
                    nc.tensor.matmul(
                        py[:, tck * 512:tck * 512 + C],
                        onesr_sb[:, :], beffr_sb[:, :],
                        start=True, stop=False, skip_group_check=True)

            def proj_pair(b, j):
                """Accumulate yct[j]^T @ wp[j] into both tck halves."""
                if b < 0 or b >= BS:
                    return
                st = state[b]
                py = st["py"]
                for tck in range(2):
                    nc.tensor.matmul(
                        py[:, tck * 512:tck * 512 + C],
                        st["yct"][j][:, tck * 128:(tck + 1) * 128],
                        wp_sb[:, j * C:(j + 1) * C],
                        start=False, stop=(j == 2),
                        skip_group_check=True,
                    )

            def proj_evac(b, tck):
                if b < 0 or b >= BS:
                    return
                st = state[b]
                py = st["py"]
                if tck == 0:
                    ysb = p_ysb.tile([128, 768], BF16, tag="ysb",
                                     name=f"ysb{b}")
                    st["ysb"] = ysb
                ysb = st["ysb"]
                nc.scalar.activation(
                    out=ysb[:, tck * 384:(tck + 1) * 384],
                    in_=py[:, tck * 512:tck * 512 + 384],
                    func=AF.Copy)
                if tck == 1:
                    del st["py"]

            def proj_store(b, tck):
                if b < 0 or b >= BS:
                    return
                st = state[b]
                ysb = st["ysb"]
                q = nc.sync if tck == 0 else nc.scalar
                q.dma_start(
                    out=y[b, tck * 128:(tck + 1) * 128, :],
                    in_=ysb[:, tck * 384:(tck + 1) * 384],
                )
                if tck == 1:
                    del state[b]

            # ---- pipeline ----
            load_xts(0)
            load_xts(1)
            for piece in range(4):
                qkv_piece(0, piece)
            load_late_consts()
            for b in range(BS):
                last = b == BS - 1
                heads_piece(b, 0, 0)
                qkv_piece(b + 1, 0)
                proj_open(b - 1)
                proj_pair(b - 1, 0)
                heads_piece(b, 0, 1)
                if last:
                    proj_open(b)
                heads_piece(b, 1, 0)
                qkv_piece(b + 1, 1)
                proj_pair(b - 1, 1)
                proj_pair(b - 1, 2)
                heads_piece(b, 1, 1)
                heads_norm(b, 0)
                if last:
                    proj_pair(b, 0)
                heads_piece(b, 2, 0)
                proj_evac(b - 1, 0)
                proj_evac(b - 1, 1)
                qkv_piece(b + 1, 2)
                heads_norm(b, 1)
                qkv_piece(b + 1, 3)
                proj_store(b - 1, 0)
                proj_store(b - 1, 1)
                load_xts(b + 2)
                heads_piece(b, 2, 1)
                heads_norm(b, 2)
                if last:
                    proj_pair(b, 1)
                    proj_pair(b, 2)
                    proj_evac(b, 0)
                    proj_evac(b, 1)
                    proj_store(b, 0)
                    proj_store(b, 1)

    nc.compile()
    return nc


def _get_compiled():
    global _compiled
    if _compiled is None:
        _compiled = _build()
    return _compiled


def _make_in_maps(x, W_qkv, b_qkv, W_proj, b_proj):
    import ml_dtypes

    F8 = ml_dtypes.float8_e4m3
    BF = ml_dtypes.bfloat16

    x = np.asarray(x, dtype=np.float32)
    W_qkv = np.asarray(W_qkv, dtype=np.float32)
    b_qkv = np.asarray(b_qkv, dtype=np.float32)
    W_proj = np.asarray(W_proj, dtype=np.float32)
    b_proj = np.asarray(b_proj, dtype=np.float32)

    # wq [512, 768] fp8: col order [q0|k0|q1|k1|q2|k2]; rows = 3
    # c-chunks + bias chunk (row 384 = q biases; k biases cancel in softmax)
    cols = []
    for p in range(3):
        cols.extend(range(p * 128, (p + 1) * 128))          # q_p
        cols.extend(range(C + p * 128, C + (p + 1) * 128))  # k_p
    wq_full = np.zeros((512, 2 * C), dtype=np.float32)
    wq_full[0:384, :] = W_qkv[:, cols]
    br = b_qkv[cols].copy()
    for p in range(3):
        br[p * 256 + 128:(p + 1) * 256] = 0.0
    wq_full[384, :] = br
    wq_f8 = wq_full.astype(F8)

    # x [BS, 512, T] fp8: 3 chunks of x^T + bias chunk [ones-row; zeros]
    xtr = np.zeros((B, 512, T), dtype=np.float32)
    xtr[:, 0:384, :] = x.transpose(0, 2, 1)
    xtr[:, 384, :] = 1.0
    xtr_f8 = xtr.astype(F8)
    # x^T again in bf16 for the V matmul
    xtb_bf = np.ascontiguousarray(x.transpose(0, 2, 1)).astype(BF)

    wv_bf = np.ascontiguousarray(W_qkv[:, 2 * C:]).astype(BF)
    wp_bf = np.ascontiguousarray(W_proj).astype(BF)

    idx = np.arange(128)
    cpack = np.zeros((128, 512), dtype=np.float32)
    cpack[:, 0:128] = np.eye(128, dtype=np.float32)
    cpack[:, 256:384] = -240.0 * (idx[None, :] < idx[:, None])
    cpack_f8 = cpack.astype(F8)
    onesb_bf = np.ones((128, 768), dtype=BF)
    beff = b_proj + b_qkv[2 * C:] @ W_proj  # v bias folded into proj bias
    bpack = np.zeros((1, 512), dtype=np.float32)
    bpack[0, 0:384] = beff
    bpack[0, 384:512] = 1.0
    bpack_bf = bpack.astype(BF)

    in_maps = []
    for c in range(N_CORES):
        in_maps.append({
            "xt": np.ascontiguousarray(xtr_f8[c * BS:(c + 1) * BS]),
            "xtb": np.ascontiguousarray(xtb_bf[c * BS:(c + 1) * BS]),
            "wq": wq_f8, "wv": wv_bf, "wp": wp_bf, "bpack": bpack_bf,
            "cpack": cpack_f8, "onesb": onesb_bf,
        })
    return in_maps


def kernel(x, W_qkv, b_qkv, W_proj, b_proj):
    nc = _get_compiled()
    from concourse.bass_utils import run_bass_kernel_spmd

    in_maps = _make_in_maps(x, W_qkv, b_qkv, W_proj, b_proj)
    res = run_bass_kernel_spmd(nc, in_maps, core_ids=list(range(N_CORES)))
    out = np.concatenate([res.results[c]["y"].astype(np.float32)
                          for c in range(N_CORES)], axis=0)
    return out


# revision 40
# speedup vs baseline: 1.1673x; 1.0290x over previous
"""Causal multi-head attention (B=64, T=256, C=384, H=6, D=64) on 8 TRN2 cores.

Data-parallel over batch (8 batches/core). Attention computed transposed
per (batch, head): S^T = K Q^T in [t, s] layout.

v2 design (fp8 DoubleRow):
  PE   : all matmuls in fp8e4m3 with DoubleRow perf mode (2 K-subtiles per
         pass, 0.5 cyc/row) except the projection (bf16 for accuracy).
         Causal masking is done ON the PE: a tiny constant matmul
         (I^T @ (-240*TRI)) accumulates -240 into the masked triangle of
         the score PSUM before exp. QKV biases ride the contraction as a
         4th "ones-row" chunk; the projection bias rides as a K=1 matmul.
  ACT  : exp (scale=0.125 folds the attention scale), v-scatter into the
         AV stationary layout, pair-2 QK evac, proj evacuation (bf16 y).
  DVE  : pair-0/1 QK evac, pvy evacuation, reciprocal of softmax sums.
  Pool : softmax normalization multiplies (SBUF-only engine).

Softmax denominators ride the AV matmul via ones-columns interleaved with
V in the stationary ([V_h | ones] per head), yielding row sums on
partitions 64:128 of the AV output.

PSUM (8 banks): ring A = 3 x [128,1024] shared by {pq01, pq2, pst x3,
pvy x3}; ring B = 1 x [128,1024] shared by {pv, py} (proj evac emitted
before the V matmuls so the buffer frees in time).
"""
import sys

for _p in ("/opt/trn_rl_repo", "/root/.axon_site/_ro/trn_rl_repo"):
    if _p not in sys.path:
        sys.path.insert(0, _p)

import numpy as np

N_CORES = 8
B, T, C = 64, 256, 384
H, D = 6, 64
BS = B // N_CORES  # batches per core

_compiled = None


def _build():
    import concourse.bass as bass
    import concourse.bacc as bacc
    import concourse.tile as tile
    from concourse import mybir

    F32 = mybir.dt.float32
    F8 = mybir.dt.float8e4
    BF16 = mybir.dt.bfloat16
    AF = mybir.ActivationFunctionType
    DR = mybir.MatmulPerfMode.DoubleRow

    nc = bacc.Bacc(None)

    # DRAM tensors
    xt = nc.dram_tensor("xt", [BS, 512, T], F8, kind="ExternalInput")
    xtb = nc.dram_tensor("xtb", [BS, 384, T], BF16, kind="ExternalInput")
    wq = nc.dram_tensor("wq", [512, 2 * C], F8, kind="ExternalInput")
    wv = nc.dram_tensor("wv", [3 * 128, C], BF16, kind="ExternalInput")
    wp = nc.dram_tensor("wp", [3 * 128, C], BF16, kind="ExternalInput")
    cpack = nc.dram_tensor("cpack", [128, 512], F8, kind="ExternalInput")
    onesb = nc.dram_tensor("onesb", [128, 768], BF16, kind="ExternalInput")
    bpack = nc.dram_tensor("bpack", [1, 512], BF16, kind="ExternalInput")
    y = nc.dram_tensor("y", [BS, T, C], BF16, kind="ExternalOutput")

    QKW = 640   # per-pair block in qk_sb: q(256) | k(256) | zeros(128)
    VAW = 1664  # vaug: 2 * 768 (sc-major, 6 x [V|ones]) + zeros(128)

    def mkap(t_, rb, np_, col, dims):
        """AP over tile t_ at partition base rb (np_ partitions), free
        offset col, extra free dims `dims` ([stride, count] pairs)."""
        full = t_[:, :]
        pstr = full.ap[0][0]
        return bass.AP(
            tensor=t_.tensor,
            offset=full.offset + rb * pstr + col,
            ap=[[pstr, np_]] + dims,
        )

    with tile.TileContext(nc) as tc:
        with (
            tc.tile_pool(name="consts", bufs=1) as consts,
            tc.tile_pool(name="xts", bufs=4) as p_xts,
            tc.tile_pool(name="qk", bufs=2) as p_qk,
            tc.tile_pool(name="pr", bufs=6) as p_pr,
            tc.tile_pool(name="vaug", bufs=3) as p_vaug,
            tc.tile_pool(name="pvysb", bufs=6) as p_pvysb,
            tc.tile_pool(name="rbt", bufs=6) as p_rbt,
            tc.tile_pool(name="yct", bufs=3) as p_yct,
            tc.tile_pool(name="ysb", bufs=3) as p_ysb,
            tc.tile_pool(name="psP", bufs=2, space="PSUM") as ps_p,
            tc.tile_pool(name="psS", bufs=2, space="PSUM") as ps_s,
            tc.tile_pool(name="psB", bufs=1, space="PSUM") as ps_b,
        ):
            # ---- constants ----
            xts0 = p_xts.tile([128, 4 * T], F8, tag="xts", name="xts0")
            nc.sync.dma_start(
                out=xts0, in_=xt[0].rearrange("(j p) t -> p j t", p=128))
            wq_sb = consts.tile([128, 4 * 768], F8, tag="wq")
            for i in range(2):
                nc.scalar.dma_start(
                    out=wq_sb[:, i * 1536:(i + 1) * 1536],
                    in_=wq[i * 256:(i + 1) * 256].rearrange(
                        "(j p) c -> p j c", p=128))
            xtb0 = p_xts.tile([128, 3 * T], BF16, tag="xtb", name="xtb0")
            nc.sync.dma_start(
                out=xtb0, in_=xtb[0].rearrange("(j p) t -> p j t", p=128))
            cpack_sb = consts.tile([128, 512], F8, tag="cpack")
            nc.scalar.dma_start(out=cpack_sb, in_=cpack[:, :])
            maskc_sb = cpack_sb[:, 0:256]
            trim_sb = cpack_sb[:, 256:512]
            wv_sb = consts.tile([128, 3 * C], BF16, tag="wv")
            wp_sb = consts.tile([128, 3 * C], BF16, tag="wp")
            bpack_sb = consts.tile([1, 512], BF16, tag="bpack")
            beffr_sb = bpack_sb[:, 0:384]
            onesr_sb = bpack_sb[:, 384:512]

            def load_late_consts():
                nc.scalar.dma_start(out=wv_sb, in_=wv.rearrange(
                    "(j p) c -> p j c", p=128))
                nc.scalar.dma_start(out=wp_sb, in_=wp.rearrange(
                    "(j p) c -> p j c", p=128))
                nc.scalar.dma_start(out=bpack_sb, in_=bpack[:, :])

            # fixed double-buffers with constant regions initialized once
            qk_bufs = [[p_qk.tile([128, QKW], F8, tag=f"qk{p}",
                                  name=f"qkbuf{p}_{i}") for p in range(3)]
                       for i in range(2)]
            for bufs in qk_bufs:
                for t_ in bufs:
                    nc.gpsimd.memzero(t_[:, 512:QKW])
            vaug_bufs = [p_vaug.tile([128, VAW], BF16, tag="vaug",
                                     name=f"vabuf{i}") for i in range(3)]
            for i, t_ in enumerate(vaug_bufs):
                nc.gpsimd.memzero(t_[:, 1536:1664])
                dst = mkap(t_, 0, 128, 64, [[768, 2], [128, 6], [1, 64]])
                q = nc.sync if i == 0 else nc.scalar
                q.dma_start(out=dst, in_=onesb[:, :])

            # PE p-state warmup: dummy matmuls on a zeroed scratch so the
            # tensor engine reaches full clock before real work arrives.
            warm_sb = consts.tile([128, 512], F8, tag="warm")
            nc.gpsimd.memzero(warm_sb[:, :])
            pwarm = ps_s.tile([128, 512], F32, tag="S", name="pwarm")
            for _w in range(10):
                nc.tensor.matmul(pwarm[:, 0:512], warm_sb[:, 0:128],
                                 warm_sb[:, 0:512],
                                 start=True, stop=True,
                                 skip_group_check=True)

            state = {}

            def qk_tile(b):
                return qk_bufs[b % 2]

            def vaug_tile(b):
                return vaug_bufs[b % 3]

            def load_xts(b):
                if b >= BS:
                    return
                if b == 0:
                    state[0] = {"xts": xts0, "xtb": xtb0}
                    return
                xts = p_xts.tile([128, 4 * T], F8, tag="xts", name=f"xts{b}")
                nc.sync.dma_start(
                    out=xts, in_=xt[b].rearrange("(j p) t -> p j t", p=128))
                xtbt = p_xts.tile([128, 3 * T], BF16, tag="xtb",
                                  name=f"xtb{b}")
                nc.sync.dma_start(
                    out=xtbt, in_=xtb[b].rearrange("(j p) t -> p j t", p=128))
                state[b] = {"xts": xts, "xtb": xtbt}

            def qkv_piece(b, piece):
                if b >= BS:
                    return
                st = state[b]
                xts = st["xts"]
                qk = qk_tile(b)
                if piece == 0:
                    # pairs 0,1 -> two 1-bank tiles; DVE evacs
                    for p in range(2):
                        pq = ps_s.tile([128, 512], F32, tag="S",
                                       name=f"pq{p}_{b}")
                        for half in range(2):
                            col = half * 256
                            wcol = p * 256 + half * 128
                            for g in range(2):
                                nc.tensor.matmul(
                                    pq[:, col:col + 256],
                                    mkap(wq_sb, 0, 128, g * 2 * 768 + wcol,
                                         [[768, 2], [1, 128]]),
                                    mkap(xts, 0, 128, g * 2 * T,
                                         [[T, 2], [1, T]]),
                                    start=(g == 0), stop=(g == 1),
                                    perf_mode=DR, skip_group_check=True,
                                )
                        nc.vector.tensor_copy(qk[p][:, 0:512], pq)
                elif piece == 1:
                    # pair 2 -> 1-bank tile; ACT evac
                    pq = ps_s.tile([128, 512], F32, tag="S", name=f"pq2_{b}")
                    for half in range(2):
                        col = half * 256
                        wcol = 512 + half * 128
                        for g in range(2):
                            nc.tensor.matmul(
                                pq[:, col:col + 256],
                                mkap(wq_sb, 0, 128, g * 2 * 768 + wcol,
                                     [[768, 2], [1, 128]]),
                                mkap(xts, 0, 128, g * 2 * T,
                                     [[T, 2], [1, T]]),
                                start=(g == 0), stop=(g == 1),
                                perf_mode=DR, skip_group_check=True,
                            )
                    nc.scalar.activation(
                        out=qk[2][:, 0:512], in_=pq[:, 0:512], func=AF.Copy)
                elif piece == 2:
                    # V [t-chunk, 384] bf16 for sc=0,1 into B tile
                    xtbt = st["xtb"]
                    pv = ps_b.tile([128, 1024], F32, tag="B", name=f"pv{b}")
                    st["pv"] = pv
                    for sc in range(2):
                        for j in range(3):
                            nc.tensor.matmul(
                                pv[:, sc * 512:sc * 512 + 384],
                                xtbt[:, j * T + sc * 128:j * T + sc * 128
                                     + 128],
                                wv_sb[:, j * C:(j + 1) * C],
                                start=(j == 0), stop=(j == 2),
                                skip_group_check=True,
                            )
                elif piece == 3:
                    # v scatter (ACT): pv [128, 2, 6, 64] -> vaug [V|ones]
                    pv = st["pv"]
                    va = vaug_tile(b)
                    src = mkap(pv, 0, 128, 0, [[512, 2], [64, 6], [1, 64]])
                    dst = mkap(va, 0, 128, 0, [[768, 2], [128, 6], [1, 64]])
                    nc.scalar.activation(out=dst, in_=src, func=AF.Copy)
                    del st["pv"]

            def heads_scores(b, g):
                """Scores + mask + exp for head group g."""
                if b < 0 or b >= BS:
                    return
                st = state[b]
                qk = qk_tile(b)[g]
                if True:
                    pst = ps_p.tile([128, 1024], F32, tag="P",
                                    name=f"pst{b}_{g}")
                    st[f"pst{g}"] = pst
                    msta = bass.AP(
                        tensor=cpack_sb.tensor,
                        offset=maskc_sb.offset,
                        ap=[[maskc_sb.ap[0][0], 128], [128, 2], [1, 128]],
                    )
                    mmov = bass.AP(
                        tensor=cpack_sb.tensor,
                        offset=trim_sb.offset,
                        ap=[[trim_sb.ap[0][0], 128], [128, 2], [1, 128]],
                    )
                    for hh in range(2):
                        rb = 64 * hh
                        col = hh * 512
                        for tck in range(2):
                            n = 256 if tck == 0 else 128
                            ksta = mkap(qk, rb, 64,
                                        256 + tck * 128,
                                        [[256 - tck * 128, 2], [1, 128]])
                            qmov = mkap(qk, rb, 64,
                                        (0 if tck == 0 else 128),
                                        [[0, 2], [1, n]])
                            nc.tensor.matmul(
                                pst[:, col + tck * 256:col + tck * 256 + n],
                                ksta, qmov,
                                start=True, stop=False,
                                perf_mode=DR, skip_group_check=True,
                            )
                            nc.tensor.matmul(
                                pst[:, col + tck * 256:col + tck * 256 + 128],
                                msta, mmov,
                                start=False, stop=True,
                                perf_mode=DR, skip_group_check=True,
                            )
                    pr = p_pr.tile([128, 768], BF16, tag="pr",
                                   name=f"pr{b}_{g}")
                    st[f"pr{g}"] = pr
                    nc.scalar.activation(
                        out=mkap(pr, 0, 128, 0, [[384, 2], [1, 384]]),
                        in_=mkap(pst, 0, 128, 0, [[512, 2], [1, 384]]),
                        func=AF.Exp, scale=0.125)
                    del st[f"pst{g}"]

            def heads_av(b, g):
                """AV matmuls + pvy evacuation for head group g."""
                if b < 0 or b >= BS:
                    return
                st = state[b]
                va = vaug_tile(b)
                if True:
                    pr = st[f"pr{g}"]
                    pvy = ps_s.tile([128, 512], F32, tag="S",
                                    name=f"pvy{b}_{g}")
                    for hh in range(2):
                        h = 2 * g + hh
                        col = hh * 256
                        # s-chunk 0: contract t0 only (bf16)
                        nc.tensor.matmul(
                            pvy[:, col:col + 128],
                            va[:, h * 128:h * 128 + 128],
                            pr[:, hh * 384:hh * 384 + 128],
                            start=True, stop=True,
                            skip_group_check=True,
                        )
                        # s-chunk 1: accumulate t0 then t1 (bf16)
                        nc.tensor.matmul(
                            pvy[:, col + 128:col + 256],
                            va[:, h * 128:h * 128 + 128],
                            pr[:, hh * 384 + 128:hh * 384 + 256],
                            start=True, stop=False,
                            skip_group_check=True,
                        )
                        nc.tensor.matmul(
                            pvy[:, col + 128:col + 256],
                            va[:, 768 + h * 128:768 + h * 128 + 128],
                            pr[:, hh * 384 + 256:hh * 384 + 384],
                            start=False, stop=True,
                            skip_group_check=True,
                        )
                    pvysb = p_pvysb.tile([128, 512], F32, tag="pvysb",
                                         name=f"pvysb{b}_{g}")
                    st[f"pvysb{g}"] = pvysb
                    nc.vector.tensor_copy(pvysb, pvy)
                    del st[f"pr{g}"]

            def heads_norm(b, g):
                """Reciprocal + normalization muls for head group g."""
                if b < 0 or b >= BS:
                    return
                st = state[b]
                pvysb = st[f"pvysb{g}"]
                rbt = p_rbt.tile([64, 512], F32, tag="rbt",
                                 name=f"rbt{b}_{g}")
                with nc.allow_low_precision(reason="softmax recip"):
                    nc.vector.reciprocal(out=rbt, in_=pvysb[64:128, :])
                if g == 0:
                    st["yct"] = [
                        p_yct.tile([128, T], BF16, tag=f"yct{j}",
                                   name=f"yct{b}_{j}")
                        for j in range(3)
                    ]
                yct = st["yct"][g]
                nc.gpsimd.tensor_mul(
                    yct[0:64, :], pvysb[0:64, 0:256], rbt[:, 0:256])
                nc.gpsimd.tensor_mul(
                    yct[64:128, :], pvysb[0:64, 256:512], rbt[:, 256:512])
                del st[f"pvysb{g}"]

            def proj_open(b):
                """Allocate py and ride the bias in (both tck halves)."""
                if b < 0 or b >= BS:
                    return
                st = state[b]
                py = ps_b.tile([128, 1024], F32, tag="B", name=f"py{b}")
                st["py"] = py
                for tck in range(2):
                    nc.tensor.matmul(
                        py[:, tck * 512:tck * 512 + C],
                        onesr_sb[:, :], beffr_sb[:, :],
                        start=True, stop=False, skip_group_check=True)

            def proj_pair(b, j):
                """Accumulate yct[j]^T @ wp[j] into both tck halves."""
                if b < 0 or b >= BS:
                    return
                st = state[b]
                py = st["py"]
                for tck in range(2):
                    nc.tensor.matmul(
                        py[:, tck * 512:tck * 512 + C],
                        st["yct"][j][:, tck * 128:(tck + 1) * 128],
                        wp_sb[:, j * C:(j + 1) * C],
                        start=False, stop=(j == 2),
                        skip_group_check=True,
                    )

            def proj_evac(b, tck):
                if b < 0 or b >= BS or tck == 1:
                    return
                st = state[b]
                py = st["py"]
                ysb = p_ysb.tile([128, 768], BF16, tag="ysb", name=f"ysb{b}")
                st["ysb"] = ysb
                nc.scalar.activation(
                    out=mkap(ysb, 0, 128, 0, [[384, 2], [1, 384]]),
                    in_=mkap(py, 0, 128, 0, [[512, 2], [1, 384]]),
                    func=AF.Copy)
                del st["py"]

            def proj_store(b, tck):
                if b < 0 or b >= BS:
                    return
                st = state[b]
                ysb = st["ysb"]
                q = nc.sync if tck == 0 else nc.scalar
                q.dma_start(
                    out=y[b, tck * 128:(tck + 1) * 128, :],
                    in_=ysb[:, tck * 384:(tck + 1) * 384],
                )
                if tck == 1:
                    del state[b]

            # ---- pipeline (4-deep: qkv b+1 | scores b | av/norm b-1 |
            # proj b-2) ----
            load_xts(0)
            load_xts(1)
            for piece in range(4):
                qkv_piece(0, piece)
            load_late_consts()
            for b in range(BS):
                heads_scores(b, 0)
                qkv_piece(b + 1, 0)
                load_xts(b + 2)
                heads_av(b - 1, 0)
                heads_norm(b - 1, 0)
                heads_scores(b, 1)
                qkv_piece(b + 1, 1)
                heads_av(b - 1, 1)
                proj_open(b - 2)
                proj_pair(b - 2, 0)
                proj_pair(b - 2, 1)
                proj_pair(b - 2, 2)
                proj_evac(b - 2, 0)
                proj_evac(b - 2, 1)
                heads_norm(b - 1, 1)
                heads_scores(b, 2)
                qkv_piece(b + 1, 2)
                heads_av(b - 1, 2)
                qkv_piece(b + 1, 3)
                proj_store(b - 2, 0)
                proj_store(b - 2, 1)
                heads_norm(b - 1, 2)
            # epilogue: drain batches BS-1 (heads) and BS-2, BS-1 (proj)
            b = BS - 1
            heads_av(b, 0)
            heads_norm(b, 0)
            proj_open(b - 1)
            proj_pair(b - 1, 0)
            proj_pair(b - 1, 1)
            heads_av(b, 1)
            heads_norm(b, 1)
            proj_pair(b - 1, 2)
            proj_evac(b - 1, 0)
            proj_evac(b - 1, 1)
            heads_av(b, 2)
            proj_store(b - 1, 0)
            proj_store(b - 1, 1)
            heads_norm(b, 2)
            proj_open(b)
            proj_pair(b, 0)
            proj_pair(b, 1)
            proj_pair(b, 2)
            proj_evac(b, 0)
            proj_evac(b, 1)
            proj_store(b, 0)
            proj_store(b, 1)

    nc.compile()
    return nc


def _get_compiled():
    global _compiled
    if _compiled is None:
        _compiled = _build()
    return _compiled


def _make_in_maps(x, W_qkv, b_qkv, W_proj, b_proj):
    import ml_dtypes

    F8 = ml_dtypes.float8_e4m3
    BF = ml_dtypes.bfloat16

    x = np.asarray(x, dtype=np.float32)
    W_qkv = np.asarray(W_qkv, dtype=np.float32)
    b_qkv = np.asarray(b_qkv, dtype=np.float32)
    W_proj = np.asarray(W_proj, dtype=np.float32)
    b_proj = np.asarray(b_proj, dtype=np.float32)

    # wq [512, 768] fp8: col order [q0|k0|q1|k1|q2|k2]; rows = 3
    # c-chunks + bias chunk (row 384 = q biases; k biases cancel in softmax)
    cols = []
    for p in range(3):
        cols.extend(range(p * 128, (p + 1) * 128))          # q_p
        cols.extend(range(C + p * 128, C + (p + 1) * 128))  # k_p
    wq_full = np.zeros((512, 2 * C), dtype=np.float32)
    wq_full[0:384, :] = W_qkv[:, cols]
    br = b_qkv[cols].copy()
    for p in range(3):
        br[p * 256 + 128:(p + 1) * 256] = 0.0
    wq_full[384, :] = br
    wq_f8 = wq_full.astype(F8)

    # x [BS, 512, T] fp8: 3 chunks of x^T + bias chunk [ones-row; zeros]
    xtr = np.zeros((B, 512, T), dtype=np.float32)
    xtr[:, 0:384, :] = x.transpose(0, 2, 1)
    xtr[:, 384, :] = 1.0
    xtr_f8 = xtr.astype(F8)
    # x^T again in bf16 for the V matmul
    xtb_bf = np.ascontiguousarray(x.transpose(0, 2, 1)).astype(BF)

    wv_bf = np.ascontiguousarray(W_qkv[:, 2 * C:]).astype(BF)
    wp_bf = np.ascontiguousarray(W_proj).astype(BF)

    idx = np.arange(128)
    cpack = np.zeros((128, 512), dtype=np.float32)
    cpack[:, 0:128] = np.eye(128, dtype=np.float32)
    cpack[:, 256:384] = -240.0 * (idx[None, :] < idx[:, None])
    cpack_f8 = cpack.astype(F8)
    onesb_bf = np.ones((128, 768), dtype=BF)
    beff = b_proj + b_qkv[2 * C:] @ W_proj  # v bias folded into proj bias
    bpack = np.zeros((1, 512), dtype=np.float32)
    bpack[0, 0:384] = beff
    bpack[0, 384:512] = 1.0
    bpack_bf = bpack.astype(BF)

    in_maps = []
    for c in range(N_CORES):
        in_maps.append({
            "xt": np.ascontiguousarray(xtr_f8[c * BS:(c + 1) * BS]),
            "xtb": np.ascontiguousarray(xtb_bf[c * BS:(c + 1) * BS]),
            "wq": wq_f8, "wv": wv_bf, "wp": wp_bf, "bpack": bpack_bf,
            "cpack": cpack_f8, "onesb": onesb_bf,
        })
    return in_maps


def kernel(x, W_qkv, b_qkv, W_proj, b_proj):
    nc = _get_compiled()
    from concourse.bass_utils import run_bass_kernel_spmd

    in_maps = _make_in_maps(x, W_qkv, b_qkv, W_proj, b_proj)
    res = run_bass_kernel_spmd(nc, in_maps, core_ids=list(range(N_CORES)))
    out = np.concatenate([res.results[c]["y"].astype(np.float32)
                          for c in range(N_CORES)], axis=0)
    return out


# revision 43
# speedup vs baseline: 1.2034x; 1.0309x over previous
"""Causal multi-head attention (B=64, T=256, C=384, H=6, D=64) on 8 TRN2 cores.

Data-parallel over batch (8 batches/core). Attention computed transposed
per (batch, head): S^T = K Q^T in [t, s] layout.

v2 design (fp8 DoubleRow):
  PE   : all matmuls in fp8e4m3 with DoubleRow perf mode (2 K-subtiles per
         pass, 0.5 cyc/row) except the projection (bf16 for accuracy).
         Causal masking is done ON the PE: a tiny constant matmul
         (I^T @ (-240*TRI)) accumulates -240 into the masked triangle of
         the score PSUM before exp. QKV biases ride the contraction as a
         4th "ones-row" chunk; the projection bias rides as a K=1 matmul.
  ACT  : exp (scale=0.125 folds the attention scale), v-scatter into the
         AV stationary layout, pair-2 QK evac, proj evacuation (bf16 y).
  DVE  : pair-0/1 QK evac, pvy evacuation, reciprocal of softmax sums.
  Pool : softmax normalization multiplies (SBUF-only engine).

Softmax denominators ride the AV matmul via ones-columns interleaved with
V in the stationary ([V_h | ones] per head), yielding row sums on
partitions 64:128 of the AV output.

PSUM (8 banks): ring A = 3 x [128,1024] shared by {pq01, pq2, pst x3,
pvy x3}; ring B = 1 x [128,1024] shared by {pv, py} (proj evac emitted
before the V matmuls so the buffer frees in time).
"""
import sys

for _p in ("/opt/trn_rl_repo", "/root/.axon_site/_ro/trn_rl_repo"):
    if _p not in sys.path:
        sys.path.insert(0, _p)

import numpy as np

N_CORES = 8
B, T, C = 64, 256, 384
H, D = 6, 64
BS = B // N_CORES  # batches per core

_compiled = None


def _build():
    import concourse.bass as bass
    import concourse.bacc as bacc
    import concourse.tile as tile
    from concourse import mybir

    F32 = mybir.dt.float32
    F8 = mybir.dt.float8e4
    BF16 = mybir.dt.bfloat16
    AF = mybir.ActivationFunctionType
    DR = mybir.MatmulPerfMode.DoubleRow

    nc = bacc.Bacc(None)

    # DRAM tensors
    xt = nc.dram_tensor("xt", [BS, 512, T], F8, kind="ExternalInput")
    xtb = nc.dram_tensor("xtb", [BS, 384, T], BF16, kind="ExternalInput")
    wq = nc.dram_tensor("wq", [512, 2 * C], F8, kind="ExternalInput")
    wv = nc.dram_tensor("wv", [3 * 128, C], BF16, kind="ExternalInput")
    wp = nc.dram_tensor("wp", [3 * 128, C], BF16, kind="ExternalInput")
    cpack = nc.dram_tensor("cpack", [128, 512], F8, kind="ExternalInput")
    onesb = nc.dram_tensor("onesb", [128, 768], BF16, kind="ExternalInput")
    bpack = nc.dram_tensor("bpack", [1, 512], BF16, kind="ExternalInput")
    y = nc.dram_tensor("y", [BS, T, C], BF16, kind="ExternalOutput")

    QKW = 640   # per-pair block in qk_sb: q(256) | k(256) | zeros(128)
    VAW = 1664  # vaug: 2 * 768 (sc-major, 6 x [V|ones]) + zeros(128)

    def mkap(t_, rb, np_, col, dims):
        """AP over tile t_ at partition base rb (np_ partitions), free
        offset col, extra free dims `dims` ([stride, count] pairs)."""
        full = t_[:, :]
        pstr = full.ap[0][0]
        return bass.AP(
            tensor=t_.tensor,
            offset=full.offset + rb * pstr + col,
            ap=[[pstr, np_]] + dims,
        )

    with tile.TileContext(nc) as tc:
        with (
            tc.tile_pool(name="consts", bufs=1) as consts,
            tc.tile_pool(name="xts", bufs=4) as p_xts,
            tc.tile_pool(name="qk", bufs=2) as p_qk,
            tc.tile_pool(name="pr", bufs=6) as p_pr,
            tc.tile_pool(name="vaug", bufs=3) as p_vaug,
            tc.tile_pool(name="pvysb", bufs=6) as p_pvysb,
            tc.tile_pool(name="rbt", bufs=6) as p_rbt,
            tc.tile_pool(name="yct", bufs=3) as p_yct,
            tc.tile_pool(name="ysb", bufs=3) as p_ysb,
            tc.tile_pool(name="psP", bufs=2, space="PSUM") as ps_p,
            tc.tile_pool(name="psS", bufs=2, space="PSUM") as ps_s,
            tc.tile_pool(name="psB", bufs=1, space="PSUM") as ps_b,
        ):
            # ---- constants ----
            xts0 = p_xts.tile([128, 4 * T], F8, tag="xts", name="xts0")
            nc.sync.dma_start(
                out=xts0, in_=xt[0].rearrange("(j p) t -> p j t", p=128))
            wq_sb = consts.tile([128, 4 * 768], F8, tag="wq")
            for i in range(2):
                nc.scalar.dma_start(
                    out=wq_sb[:, i * 1536:(i + 1) * 1536],
                    in_=wq[i * 256:(i + 1) * 256].rearrange(
                        "(j p) c -> p j c", p=128))
            xtb0 = p_xts.tile([128, 3 * T], BF16, tag="xtb", name="xtb0")
            nc.sync.dma_start(
                out=xtb0, in_=xtb[0].rearrange("(j p) t -> p j t", p=128))
            wv_sb = consts.tile([128, 3 * C], BF16, tag="wv")
            nc.scalar.dma_start(out=wv_sb, in_=wv.rearrange(
                "(j p) c -> p j c", p=128))
            cpack_sb = consts.tile([128, 512], F8, tag="cpack")
            nc.scalar.dma_start(out=cpack_sb, in_=cpack[:, :])
            maskc_sb = cpack_sb[:, 0:256]
            trim_sb = cpack_sb[:, 256:512]
            wp_sb = consts.tile([128, 3 * C], BF16, tag="wp")
            bpack_sb = consts.tile([1, 512], BF16, tag="bpack")
            beffr_sb = bpack_sb[:, 0:384]
            onesr_sb = bpack_sb[:, 384:512]

            def load_late_consts():
                nc.scalar.dma_start(out=wp_sb, in_=wp.rearrange(
                    "(j p) c -> p j c", p=128))
                nc.scalar.dma_start(out=bpack_sb, in_=bpack[:, :])

            # fixed double-buffers with constant regions initialized once
            qk_bufs = [[p_qk.tile([128, QKW], F8, tag=f"qk{p}",
                                  name=f"qkbuf{p}_{i}") for p in range(3)]
                       for i in range(2)]
            for bufs in qk_bufs:
                for t_ in bufs:
                    nc.gpsimd.memzero(t_[:, 512:QKW])
            vaug_bufs = [p_vaug.tile([128, VAW], BF16, tag="vaug",
                                     name=f"vabuf{i}") for i in range(3)]
            for i, t_ in enumerate(vaug_bufs):
                nc.gpsimd.memzero(t_[:, 1536:1664])
                dst = mkap(t_, 0, 128, 64, [[768, 2], [128, 6], [1, 64]])
                q = nc.sync if i == 0 else nc.scalar
                q.dma_start(out=dst, in_=onesb[:, :])

            # PE p-state warmup: dummy matmuls on a zeroed scratch so the
            # tensor engine reaches full clock before real work arrives.
            warm_sb = consts.tile([128, 512], F8, tag="warm")
            nc.gpsimd.memzero(warm_sb[:, :])
            pwarm = ps_s.tile([128, 512], F32, tag="S", name="pwarm")
            for _w in range(10):
                nc.tensor.matmul(pwarm[:, 0:512], warm_sb[:, 0:128],
                                 warm_sb[:, 0:512],
                                 start=True, stop=True,
                                 skip_group_check=True)

            state = {}

            def qk_tile(b):
                return qk_bufs[b % 2]

            def vaug_tile(b):
                return vaug_bufs[b % 3]

            def load_xts(b):
                if b >= BS:
                    return
                if b == 0:
                    state[0] = {"xts": xts0, "xtb": xtb0}
                    return
                xts = p_xts.tile([128, 4 * T], F8, tag="xts", name=f"xts{b}")
                nc.sync.dma_start(
                    out=xts, in_=xt[b].rearrange("(j p) t -> p j t", p=128))
                state[b] = {"xts": xts}

            def load_xtb(b):
                if b >= BS or b == 0:
                    return
                xtbt = p_xts.tile([128, 3 * T], BF16, tag="xtb",
                                  name=f"xtb{b}")
                nc.sync.dma_start(
                    out=xtbt, in_=xtb[b].rearrange("(j p) t -> p j t", p=128))
                state[b]["xtb"] = xtbt

            def qkv_piece(b, piece):
                if b >= BS:
                    return
                st = state[b]
                xts = st["xts"]
                qk = qk_tile(b)
                if piece == 0:
                    # pairs 0,1 -> two 1-bank tiles; DVE evacs
                    for p in range(2):
                        pq = ps_s.tile([128, 512], F32, tag="S",
                                       name=f"pq{p}_{b}")
                        for half in range(2):
                            col = half * 256
                            wcol = p * 256 + half * 128
                            for g in range(2):
                                nc.tensor.matmul(
                                    pq[:, col:col + 256],
                                    mkap(wq_sb, 0, 128, g * 2 * 768 + wcol,
                                         [[768, 2], [1, 128]]),
                                    mkap(xts, 0, 128, g * 2 * T,
                                         [[T, 2], [1, T]]),
                                    start=(g == 0), stop=(g == 1),
                                    perf_mode=DR, skip_group_check=True,
                                )
                        nc.vector.tensor_copy(qk[p][:, 0:512], pq)
                elif piece == 1:
                    # pair 2 -> 1-bank tile; ACT evac
                    pq = ps_s.tile([128, 512], F32, tag="S", name=f"pq2_{b}")
                    for half in range(2):
                        col = half * 256
                        wcol = 512 + half * 128
                        for g in range(2):
                            nc.tensor.matmul(
                                pq[:, col:col + 256],
                                mkap(wq_sb, 0, 128, g * 2 * 768 + wcol,
                                     [[768, 2], [1, 128]]),
                                mkap(xts, 0, 128, g * 2 * T,
                                     [[T, 2], [1, T]]),
                                start=(g == 0), stop=(g == 1),
                                perf_mode=DR, skip_group_check=True,
                            )
                    nc.scalar.activation(
                        out=qk[2][:, 0:512], in_=pq[:, 0:512], func=AF.Copy)
                elif piece == 2:
                    # V [t-chunk, 384] bf16 for sc=0,1 into B tile
                    xtbt = st["xtb"]
                    pv = ps_b.tile([128, 1024], F32, tag="B", name=f"pv{b}")
                    st["pv"] = pv
                    for sc in range(2):
                        for j in range(3):
                            nc.tensor.matmul(
                                pv[:, sc * 512:sc * 512 + 384],
                                xtbt[:, j * T + sc * 128:j * T + sc * 128
                                     + 128],
                                wv_sb[:, j * C:(j + 1) * C],
                                start=(j == 0), stop=(j == 2),
                                skip_group_check=True,
                            )
                elif piece == 3:
                    # v scatter (ACT): pv [128, 2, 6, 64] -> vaug [V|ones]
                    pv = st["pv"]
                    va = vaug_tile(b)
                    src = mkap(pv, 0, 128, 0, [[512, 2], [64, 6], [1, 64]])
                    dst = mkap(va, 0, 128, 0, [[768, 2], [128, 6], [1, 64]])
                    nc.scalar.activation(out=dst, in_=src, func=AF.Copy)
                    del st["pv"]

            def heads_scores(b, g):
                """Scores + mask + exp for head group g."""
                if b < 0 or b >= BS:
                    return
                st = state[b]
                qk = qk_tile(b)[g]
                if True:
                    pst = ps_p.tile([128, 1024], F32, tag="P",
                                    name=f"pst{b}_{g}")
                    st[f"pst{g}"] = pst
                    msta = bass.AP(
                        tensor=cpack_sb.tensor,
                        offset=maskc_sb.offset,
                        ap=[[maskc_sb.ap[0][0], 128], [128, 2], [1, 128]],
                    )
                    mmov = bass.AP(
                        tensor=cpack_sb.tensor,
                        offset=trim_sb.offset,
                        ap=[[trim_sb.ap[0][0], 128], [128, 2], [1, 128]],
                    )
                    for hh in range(2):
                        rb = 64 * hh
                        col = hh * 512
                        for tck in range(2):
                            n = 256 if tck == 0 else 128
                            ksta = mkap(qk, rb, 64,
                                        256 + tck * 128,
                                        [[256 - tck * 128, 2], [1, 128]])
                            qmov = mkap(qk, rb, 64,
                                        (0 if tck == 0 else 128),
                                        [[0, 2], [1, n]])
                            nc.tensor.matmul(
                                pst[:, col + tck * 256:col + tck * 256 + n],
                                ksta, qmov,
                                start=True, stop=False,
                                perf_mode=DR, skip_group_check=True,
                            )
                            nc.tensor.matmul(
                                pst[:, col + tck * 256:col + tck * 256 + 128],
                                msta, mmov,
                                start=False, stop=True,
                                perf_mode=DR, skip_group_check=True,
                            )
                    pr = p_pr.tile([128, 768], BF16, tag="pr",
                                   name=f"pr{b}_{g}")
                    st[f"pr{g}"] = pr
                    nc.scalar.activation(
                        out=mkap(pr, 0, 128, 0, [[384, 2], [1, 384]]),
                        in_=mkap(pst, 0, 128, 0, [[512, 2], [1, 384]]),
                        func=AF.Exp, scale=0.125)
                    del st[f"pst{g}"]

            def heads_av(b, g):
                """AV matmuls + pvy evacuation for head group g."""
                if b < 0 or b >= BS:
                    return
                st = state[b]
                va = vaug_tile(b)
                if True:
                    pr = st[f"pr{g}"]
                    pvy = ps_s.tile([128, 512], F32, tag="S",
                                    name=f"pvy{b}_{g}")
                    for hh in range(2):
                        h = 2 * g + hh
                        col = hh * 256
                        # s-chunk 0: contract t0 only (bf16)
                        nc.tensor.matmul(
                            pvy[:, col:col + 128],
                            va[:, h * 128:h * 128 + 128],
                            pr[:, hh * 384:hh * 384 + 128],
                            start=True, stop=True,
                            skip_group_check=True,
                        )
                        # s-chunk 1: accumulate t0 then t1 (bf16)
                        nc.tensor.matmul(
                            pvy[:, col + 128:col + 256],
                            va[:, h * 128:h * 128 + 128],
                            pr[:, hh * 384 + 128:hh * 384 + 256],
                            start=True, stop=False,
                            skip_group_check=True,
                        )
                        nc.tensor.matmul(
                            pvy[:, col + 128:col + 256],
                            va[:, 768 + h * 128:768 + h * 128 + 128],
                            pr[:, hh * 384 + 256:hh * 384 + 384],
                            start=False, stop=True,
                            skip_group_check=True,
                        )
                    pvysb = p_pvysb.tile([128, 512], F32, tag="pvysb",
                                         name=f"pvysb{b}_{g}")
                    st[f"pvysb{g}"] = pvysb
                    nc.vector.tensor_copy(pvysb, pvy)
                    del st[f"pr{g}"]

            def heads_norm(b, g):
                """Reciprocal + normalization muls for head group g."""
                if b < 0 or b >= BS:
                    return
                st = state[b]
                pvysb = st[f"pvysb{g}"]
                rbt = p_rbt.tile([64, 512], F32, tag="rbt",
                                 name=f"rbt{b}_{g}")
                with nc.allow_low_precision(reason="softmax recip"):
                    nc.vector.reciprocal(out=rbt, in_=pvysb[64:128, :])
                if g == 0:
                    st["yct"] = [
                        p_yct.tile([128, T], BF16, tag=f"yct{j}",
                                   name=f"yct{b}_{j}")
                        for j in range(3)
                    ]
                yct = st["yct"][g]
                nc.gpsimd.tensor_mul(
                    yct[0:64, :], pvysb[0:64, 0:256], rbt[:, 0:256])
                nc.gpsimd.tensor_mul(
                    yct[64:128, :], pvysb[0:64, 256:512], rbt[:, 256:512])
                del st[f"pvysb{g}"]

            def proj_open(b):
                """Allocate py and ride the bias in (both tck halves)."""
                if b < 0 or b >= BS:
                    return
                st = state[b]
                py = ps_b.tile([128, 1024], F32, tag="B", name=f"py{b}")
                st["py"] = py
                for tck in range(2):
                    nc.tensor.matmul(
                        py[:, tck * 512:tck * 512 + C],
                        onesr_sb[:, :], beffr_sb[:, :],
                        start=True, stop=False, skip_group_check=True)

            def proj_pair(b, j):
                """Accumulate yct[j]^T @ wp[j] into both tck halves."""
                if b < 0 or b >= BS:
                    return
                st = state[b]
                py = st["py"]
                for tck in range(2):
                    nc.tensor.matmul(
                        py[:, tck * 512:tck * 512 + C],
                        st["yct"][j][:, tck * 128:(tck + 1) * 128],
                        wp_sb[:, j * C:(j + 1) * C],
                        start=False, stop=(j == 2),
                        skip_group_check=True,
                    )

            def proj_evac(b, tck):
                if b < 0 or b >= BS or tck == 1:
                    return
                st = state[b]
                py = st["py"]
                ysb = p_ysb.tile([128, 768], BF16, tag="ysb", name=f"ysb{b}")
                st["ysb"] = ysb
                nc.scalar.activation(
                    out=mkap(ysb, 0, 128, 0, [[384, 2], [1, 384]]),
                    in_=mkap(py, 0, 128, 0, [[512, 2], [1, 384]]),
                    func=AF.Copy)
                del st["py"]

            def proj_store(b, tck):
                if b < 0 or b >= BS:
                    return
                st = state[b]
                ysb = st["ysb"]
                q = nc.sync if tck == 0 else nc.scalar
                q.dma_start(
                    out=y[b, tck * 128:(tck + 1) * 128, :],
                    in_=ysb[:, tck * 384:(tck + 1) * 384],
                )
                if tck == 1:
                    del state[b]

            # ---- pipeline (4-deep: qkv b+1 | scores b | av/norm b-1 |
            # proj b-2) ----
            load_xts(0)
            load_xts(1)
            for piece in range(4):
                qkv_piece(0, piece)
            load_late_consts()
            for b in range(BS):
                heads_scores(b, 0)
                qkv_piece(b + 1, 0)
                load_xts(b + 2)
                heads_av(b - 1, 0)
                heads_norm(b - 1, 0)
                heads_scores(b, 1)
                qkv_piece(b + 1, 1)
                load_xtb(b + 1)
                heads_av(b - 1, 1)
                proj_open(b - 2)
                proj_pair(b - 2, 0)
                proj_pair(b - 2, 1)
                proj_pair(b - 2, 2)
                proj_evac(b - 2, 0)
                proj_evac(b - 2, 1)
                heads_norm(b - 1, 1)
                heads_scores(b, 2)
                qkv_piece(b + 1, 2)
                heads_av(b - 1, 2)
                qkv_piece(b + 1, 3)
                proj_store(b - 2, 0)
                proj_store(b - 2, 1)
                heads_norm(b - 1, 2)
            # epilogue: drain batches BS-1 (heads) and BS-2, BS-1 (proj)
            b = BS - 1
            heads_av(b, 0)
            heads_norm(b, 0)
            proj_open(b - 1)
            proj_pair(b - 1, 0)
            proj_pair(b - 1, 1)
            heads_av(b, 1)
            heads_norm(b, 1)
            proj_pair(b - 1, 2)
            proj_evac(b - 1, 0)
            proj_evac(b - 1, 1)
            heads_av(b, 2)
            proj_store(b - 1, 0)
            proj_store(b - 1, 1)
            heads_norm(b, 2)
            proj_open(b)
            proj_pair(b, 0)
            proj_pair(b, 1)
            proj_pair(b, 2)
            proj_evac(b, 0)
            proj_evac(b, 1)
            proj_store(b, 0)
            proj_store(b, 1)

    nc.compile()
    return nc


def _get_compiled():
    global _compiled
    if _compiled is None:
        _compiled = _build()
    return _compiled


def _make_in_maps(x, W_qkv, b_qkv, W_proj, b_proj):
    import ml_dtypes

    F8 = ml_dtypes.float8_e4m3
    BF = ml_dtypes.bfloat16

    x = np.asarray(x, dtype=np.float32)
    W_qkv = np.asarray(W_qkv, dtype=np.float32)
    b_qkv = np.asarray(b_qkv, dtype=np.float32)
    W_proj = np.asarray(W_proj, dtype=np.float32)
    b_proj = np.asarray(b_proj, dtype=np.float32)

    # wq [512, 768] fp8: col order [q0|k0|q1|k1|q2|k2]; rows = 3
    # c-chunks + bias chunk (row 384 = q biases; k biases cancel in softmax)
    cols = []
    for p in range(3):
        cols.extend(range(p * 128, (p + 1) * 128))          # q_p
        cols.extend(range(C + p * 128, C + (p + 1) * 128))  # k_p
    wq_full = np.zeros((512, 2 * C), dtype=np.float32)
    wq_full[0:384, :] = W_qkv[:, cols]
    br = b_qkv[cols].copy()
    for p in range(3):
        br[p * 256 + 128:(p + 1) * 256] = 0.0
    wq_full[384, :] = br
    wq_f8 = wq_full.astype(F8)

    # x [BS, 512, T] fp8: 3 chunks of x^T + bias chunk [ones-row; zeros]
    xtr = np.zeros((B, 512, T), dtype=np.float32)
    xtr[:, 0:384, :] = x.transpose(0, 2, 1)
    xtr[:, 384, :] = 1.0
    xtr_f8 = xtr.astype(F8)
    # x^T again in bf16 for the V matmul
    xtb_bf = np.ascontiguousarray(x.transpose(0, 2, 1)).astype(BF)

    wv_bf = np.ascontiguousarray(W_qkv[:, 2 * C:]).astype(BF)
    wp_bf = np.ascontiguousarray(W_proj).astype(BF)

    idx = np.arange(128)
    cpack = np.zeros((128, 512), dtype=np.float32)
    cpack[:, 0:128] = np.eye(128, dtype=np.float32)
    cpack[:, 256:384] = -240.0 * (idx[None, :] < idx[:, None])
    cpack_f8 = cpack.astype(F8)
    onesb_bf = np.ones((128, 768), dtype=BF)
    beff = b_proj + b_qkv[2 * C:] @ W_proj  # v bias folded into proj bias
    bpack = np.zeros((1, 512), dtype=np.float32)
    bpack[0, 0:384] = beff
    bpack[0, 384:512] = 1.0
    bpack_bf = bpack.astype(BF)

    in_maps = []
    for c in range(N_CORES):
        in_maps.append({
            "xt": np.ascontiguousarray(xtr_f8[c * BS:(c + 1) * BS]),
            "xtb": np.ascontiguousarray(xtb_bf[c * BS:(c + 1) * BS]),
            "wq": wq_f8, "wv": wv_bf, "wp": wp_bf, "bpack": bpack_bf,
            "cpack": cpack_f8, "onesb": onesb_bf,
        })
    return in_maps


def kernel(x, W_qkv, b_qkv, W_proj, b_proj):
    nc = _get_compiled()
    from concourse.bass_utils import run_bass_kernel_spmd

    in_maps = _make_in_maps(x, W_qkv, b_qkv, W_proj, b_proj)
    res = run_bass_kernel_spmd(nc, in_maps, core_ids=list(range(N_CORES)))
    out = np.concatenate([res.results[c]["y"].astype(np.float32)
                          for c in range(N_CORES)], axis=0)
    return out
